# revision 1
# baseline (speedup 1.0000x reference)
"""Trainium2 Bass kernel for nn_BiSNN (BiSNN forward, batch-parallel over 8 cores).

Math (per sample b):
  x_feat = mean(x[b], spatial)                      (C=64,)
  h = relu(BN1(x_feat @ w_in.T))                    (HID=256,)
  PLIF recurrence, T=4: mem = d*(mem - vth*sp) + h; mem /= mean|mem|+1e-6;
                        sp = (mem >= vth)
  binary = 2*sp - 1;  mod = 1 + 0.5*tanh(scale * (binary @ w_out.T))   (C,)
  spatial map is constant per (b,c)  =>  depthwise 3x3 conv of a constant
  map has only 9 distinct outputs per (b,c): v * S[c, a, s] where S is the
  window-sum of conv_w over the valid part of the 3x3 window.
  out = 1 + 0.25*tanh(relu(BN2(v * S)))  -> 9 values per (b,c), broadcast
  into the (112,112) image.

Device schedule per core (8 samples = 4 sample-PAIRS of 128 dram rows):
  - x is staged in DRAM as float16 and streamed on the sync HWDGE ring in
    ~1.3-2.3us column slices; row sums run sliced on DVE (~1.08 ns/elem)
    and ACT copy+accum (~0.89 ns/elem) so the list scheduler can backfill
    slices into SNN-chain dependency bubbles without long head-of-line
    blocking.  All four pairs' reads+reduces are emitted ahead of the
    SNN groups so the last pair's slices win scheduler ties.  Partials
    fold with one tiny reduce into a block lhsT [128,8] (a private
    column pair per sample pair -- no aliasing, so emission order adds
    no false deps) whose zero structure routes each sample half; one
    [K=128,M,256] matmul (plus a rank-1 bias matmul) then gives h
    directly, no shuffle DMA.  d and v_th are baked as compile-time
    immediates (kernel is rebuilt if they change).
  - The serial PLIF chain runs batched for pairs {0,1} and {2,3} on
    [4,256] tiles (engine cost halves; the chain is latency-bound, ~28
    small DVE ops).  Output projection: PE transposes + a block-diagonal
    w_out matmul with strided rhs yield (128,|group|) mod values.
  - Per pair, the 9-value table (uint8 quant units, +0.5 rounding baked
    in) expands to a [128, 112+1232+112] uint8 pattern: 2 broadcast
    copies per distinct row + a tiny SBUF->SBUF stride-0 replicate DMA
    for the other 10 interior rows.  Output writes are pattern-repeat
    DMAs (stride-0 source, 1232B descriptors) on the gpsimd SWDGE ring;
    the last pair splits its repeat across the then-idle sync ring.

I/O precision: the kernel would be DMA-bound at ~420 GB/s/core combined;
f16 input (mean over 12544 pixels keeps ~4e-4 accuracy) and uint8 output
(quantizing 1+0.25*relu-tanh over [1,1.25], step 1/1020, max err ~1e-3
vs a 2e-2 tolerance; host dequantizes out = 1 + u8*0.25/255) cut traffic
from 25.7+25.7 MB to 12.8+6.4 MB per core.  At ~90us the kernel is
limited by the DVE/ACT element-rate of the row-sum reduction plus the
serial SNN chains, overlapped with the read and write streams.
"""
import os
import sys

import numpy as np

sys.path.insert(0, "/opt/trn_rl_repo")

B, C, H, W = 64, 64, 112, 112
HW = H * W          # 12544
HID = 256
T = 4
BN_EPS = 1e-5
NCORES = 8
NB = B // NCORES    # samples per core = 8
NPAIR = NB // 2     # sample pairs per core = 4
ROWS = NB * C       # 512 dram rows per core
IBLK = 11                      # interior rows materialized per block
IW = IBLK * 112                # interior block width
NRI = 110 // IBLK              # block repeats to cover rows 1..110
PAT_W = 112 + IW + 112         # rowA | interior block | rowC

_CACHE = {}
LAST_RESULTS = None


def _ensure_ntff_hook_module():
    """concourse's trace path imports antenv.axon_hooks, which the agent
    image doesn't ship; provide a ctypes-based shim so trace=True (or a
    BASS_TRACE env set by a caller) works instead of crashing."""
    try:
        import antenv.axon_hooks  # noqa: F401
        return
    except ImportError:
        pass
    import contextlib
    import ctypes
    import types

    mod = types.ModuleType("antenv.axon_hooks")
    state = {"hook": None, "tried": False}

    def _make_hook(so_path):
        lib = ctypes.CDLL(so_path)
        if not hasattr(lib, "axon_start_nrt_profile"):
            return None
        lib.axon_start_nrt_profile.argtypes = [
            ctypes.POINTER(ctypes.c_int64), ctypes.c_size_t]
        lib.axon_start_nrt_profile.restype = ctypes.c_int64
        lib.axon_stop_nrt_profile.argtypes = [ctypes.c_char_p]
        lib.axon_stop_nrt_profile.restype = ctypes.c_int64

        @contextlib.contextmanager
        def _hook(output_dir, device_ids):
            import jax
            jax.devices()
            if device_ids:
                ids = (ctypes.c_int64 * len(device_ids))(*device_ids)
                rc = lib.axon_start_nrt_profile(ids, len(device_ids))
            else:
                rc = lib.axon_start_nrt_profile(None, 0)
            if rc != 0:
                raise RuntimeError(f"axon_start_nrt_profile rc={rc}")
            try:
                yield
            finally:
                n = lib.axon_stop_nrt_profile(str(output_dir).encode())
                if n < 0:
                    raise RuntimeError(f"axon_stop_nrt_profile rc={n}")

        return _hook

    def get_axon_ntff_profile_hook():
        if state["hook"] is None and not state["tried"]:
            state["tried"] = True
            so = "/opt/axon/libaxon_pjrt.so"
            if os.path.exists(so):
                try:
                    state["hook"] = _make_hook(so)
                except OSError:
                    state["hook"] = None
        return state["hook"]

    def set_axon_ntff_profile_hook(hook):
        state["hook"] = hook
        state["tried"] = True

    mod.get_axon_ntff_profile_hook = get_axon_ntff_profile_hook
    mod.set_axon_ntff_profile_hook = set_axon_ntff_profile_hook
    sys.modules["antenv.axon_hooks"] = mod


def _emit(tc, aps, dvals):
    import concourse.bass as bass
    from concourse import mybir

    nc = tc.nc
    f32 = mybir.dt.float32
    f16 = mybir.dt.float16
    u8 = mybir.dt.uint8
    AF = mybir.ActivationFunctionType
    OP = mybir.AluOpType
    AX = mybir.AxisListType

    d, vth = dvals["d"], dvals["vth"]   # compile-time immediates

    xs, w_in_dup, b1row, w_out4, scale128, s2b2, ident4, out = (
        aps["xs"], aps["w_in_dup"], aps["b1row"], aps["w_out4"],
        aps["scale128"], aps["s2b2"], aps["ident4"], aps["out"])

    ctx = tc._emit_ctx
    cpool = ctx.enter_context(tc.tile_pool(name="consts", bufs=1))
    xpool = ctx.enter_context(tc.tile_pool(name="xin", bufs=4))
    spool = ctx.enter_context(tc.tile_pool(name="small", bufs=1))
    ppool = ctx.enter_context(tc.tile_pool(name="ps", bufs=2, space="PSUM"))

    # ---- tiny params on the gpsimd (SWDGE) ring ----
    w_in_sb = cpool.tile([128, HID], f32)
    nc.gpsimd.dma_start(w_in_sb[:], w_in_dup[:])
    b1_sb = cpool.tile([1, HID], f32)
    nc.gpsimd.dma_start(b1_sb[:], b1row[:])
    w_out_sb = cpool.tile([128, 512], f32)
    nc.gpsimd.dma_start(w_out_sb[:], w_out4[:])
    scale_sb = cpool.tile([128, 1], f32)
    nc.gpsimd.dma_start(scale_sb[:], scale128[:])
    s2b2_sb = cpool.tile([128, 18], f32)
    nc.gpsimd.dma_start(s2b2_sb[:], s2b2[:])
    id4_sb = cpool.tile([4, 4], f32)
    nc.gpsimd.dma_start(id4_sb[:], ident4[:])

    ones14 = cpool.tile([1, 4], f32)
    nc.vector.memset(ones14[:], 1.0)
    # block lhsT for the h matmul: col 2*s+half <- pair-s sums on
    # partitions half*64..half*64+64, zeros elsewhere (set once).  One
    # column pair per PAIR (no reuse across groups), so read/reduce
    # emission order carries no false dependencies on it.
    lhsT128 = spool.tile([128, 2 * NPAIR], f32)
    nc.vector.memset(lhsT128[:], 0.0)

    # reduce work is sliced into ~1.5us pieces alternating ACT/DVE: the
    # list scheduler back-fills ready pieces into dependency bubbles of
    # the serial SNN chain, and small pieces keep that head-of-line
    # blocking ~1.5us instead of ~4us.  ACT takes ~2/3 of the elements
    # (0.89 vs 1.08 ns/elem, and DVE owns the SNN glue).
    scratch = spool.tile([128, 10400], f16)

    state = {}

    SPLITS = [(0, 2900, "act"), (2900, 4875, "dve"), (4875, 7800, "act"),
              (7800, 9775, "dve"), (9775, HW, "act")]
    SPLITS_LAST = SPLITS[:-1] + [(9775, 11500, "act"), (11500, HW, "dve")]

    def stage_read(s):
        xt = xpool.tile([128, HW], f16, tag="xt")
        r0 = 128 * s
        splits = SPLITS_LAST if s == NPAIR - 1 else SPLITS
        psum = spool.tile([128, len(splits)], f32, tag=f"psum_{s}")
        for c0, c1, eng in splits:
            nc.sync.dma_start(xt[:, c0:c1], xs[r0:r0 + 128, c0:c1])
        sco = 0
        for ci, (c0, c1, eng) in enumerate(splits):
            if eng == "dve":
                nc.vector.reduce_sum(out=psum[:, ci:ci + 1],
                                     in_=xt[:, c0:c1], axis=AX.X)
            else:
                nc.scalar.activation(scratch[:, sco:sco + c1 - c0],
                                     xt[:, c0:c1], AF.Copy,
                                     accum_out=psum[:, ci:ci + 1])
                sco += c1 - c0
        # fold the partials with one tiny reduce, then park each half in
        # its block-lhsT column
        sums = spool.tile([128, 1], f32, tag=f"sums{s}")
        nc.vector.reduce_sum(out=sums[:], in_=psum[:], axis=AX.X)
        cb = 2 * s
        nc.vector.tensor_copy(lhsT128[0:64, cb:cb + 1], sums[0:64, :])
        nc.vector.tensor_copy(lhsT128[64:128, cb + 1:cb + 2],
                              sums[64:128, :])

    def stage_group(g, prs):
        S = 2 * len(prs)
        cb = 2 * prs[0]
        # ---- h = relu(w_in.T @ x_feat + b1), whole group in one matmul
        h_ps4 = ppool.tile([4, HID], f32, tag="ps_h")
        h_ps = h_ps4[0:S, :]
        nc.tensor.matmul(h_ps, lhsT=lhsT128[:, cb:cb + S],
                         rhs=w_in_sb[:], start=True, stop=False)
        nc.tensor.matmul(h_ps, lhsT=ones14[0:1, 0:S],
                         rhs=b1_sb[0:1, :], start=False, stop=True)
        h = spool.tile([S, HID], f32, tag=f"h{g}")
        nc.vector.tensor_scalar(out=h[:], in0=h_ps, scalar1=0.0,
                                scalar2=None, op0=OP.max)

        # ---- PLIF recurrence (normalization folded into the next-step
        # decay); d and v_th are compile-time immediates ----
        mem = spool.tile([S, HID], f32, tag=f"mem{g}")
        spike = spool.tile([S, HID], f32, tag=f"spike{g}")
        q = spool.tile([S, HID], f32, tag=f"q{g}")
        den = spool.tile([S, 5], f32, tag=f"den{g}")
        src = h
        for t in range(T):
            if t > 0:
                nc.vector.scalar_tensor_tensor(
                    out=q[:], in0=spike[:], scalar=-d * vth, in1=h[:],
                    op0=OP.mult, op1=OP.add)
                nc.vector.scalar_tensor_tensor(
                    out=mem[:], in0=src[:], scalar=den[:, 4:5], in1=q[:],
                    op0=OP.mult, op1=OP.add)
                src = mem
            nc.vector.reduce_sum(out=den[:, 0:1], in_=src[:], axis=AX.X,
                                 apply_absolute_value=True)
            nc.vector.tensor_scalar(out=den[:, 3:4], in0=den[:, 0:1],
                                    scalar1=vth / HID, scalar2=vth * 1e-6,
                                    op0=OP.mult, op1=OP.add)
            nc.vector.tensor_scalar(out=spike[:], in0=src[:],
                                    scalar1=den[:, 3:4],
                                    scalar2=None, op0=OP.is_ge)
            if t < T - 1:
                nc.vector.tensor_scalar(out=den[:, 1:2], in0=den[:, 0:1],
                                        scalar1=1.0 / HID, scalar2=1e-6,
                                        op0=OP.mult, op1=OP.add)
                nc.vector.reciprocal(den[:, 2:3], den[:, 1:2])
                nc.vector.tensor_scalar(out=den[:, 4:5], in0=den[:, 2:3],
                                        scalar1=d, scalar2=None,
                                        op0=OP.mult)

        binary = spool.tile([S, HID], f32, tag=f"bin{g}")
        nc.vector.tensor_scalar(out=binary[:], in0=spike[:], scalar1=2.0,
                                scalar2=-1.0, op0=OP.mult, op1=OP.add)

        # ---- transpose (S,256)->(256,S): binT[:, S*k+j] = chunk k samp j
        binT = spool.tile([128, 2 * S], f32, tag=f"binT{g}")
        for k in range(2):
            tp = ppool.tile([128, 4], f32, tag="ps_t")
            nc.tensor.transpose(tp[:, 0:S], binary[:, 128 * k:128 * (k + 1)],
                                id4_sb[0:S, 0:S])
            nc.vector.tensor_copy(binT[:, S * k:S * (k + 1)], tp[:, 0:S])

        # block-diag matmul -> mp_ps[128, len(prs)]: strided rhs picks the
        # even/odd samples of each pair per w_out4 block
        mp_ps2 = ppool.tile([128, 2], f32, tag="ps_m")
        mp_ps = mp_ps2[:, 0:len(prs)]
        for i, (wc, k, par) in enumerate([(0, 0, 0), (128, 1, 0),
                                          (256, 0, 1), (384, 1, 1)]):
            b0 = S * k + par
            rhs = bass.AP(binT.tensor, binT[:, b0:b0 + 1].offset,
                          [list(binT.ap[0]), [2, len(prs)]])
            nc.tensor.matmul(mp_ps, lhsT=w_out_sb[:, wc:wc + 128],
                             rhs=rhs, start=(i == 0), stop=(i == 3))

        # ---- 9*len(prs)-value table in uint8 quant units, +0.5 baked
        t1 = spool.tile([128, len(prs)], f32, tag=f"t1{g}")
        nc.scalar.activation(t1[:], mp_ps, AF.Tanh, scale=scale_sb[:, 0:1])
        val = spool.tile([128, 9 * len(prs)], f32, tag=f"val{g}")
        for j in range(len(prs)):
            nc.vector.scalar_tensor_tensor(
                out=val[:, 9 * j:9 * j + 9], in0=s2b2_sb[:, 0:9],
                scalar=t1[:, j:j + 1], in1=s2b2_sb[:, 9:18],
                op0=OP.mult, op1=OP.add)
        nc.scalar.activation(val[:], val[:], AF.Tanh)
        nc.scalar.activation(val[:], val[:], AF.Relu, scale=255.0)
        # +0.5 makes the truncating uint8 cast of the copies round-half-up
        nc.vector.tensor_scalar(out=val[:], in0=val[:], scalar1=0.5,
                                scalar2=None, op0=OP.add)
        for j, s in enumerate(prs):
            state[s] = (val, j)

    def stage_pat(s, weng):
        val, j = state[s]

        def vofs(k):
            return val[:, 9 * j + k:9 * j + k + 1].offset

        def bc2(k, n):
            return bass.AP(val.tensor, vofs(k), [list(val.ap[0]), [0, n]])

        pat = spool.tile([128, PAT_W], u8, tag=f"pat{s}")
        co = 112 + IW

        # three distinct rows, 2 copies each (stride-0 broadcast for the
        # 110 middle columns, then both corners in one strided copy);
        # rows 2..11 of the interior block come from a tiny SBUF->SBUF
        # stride-0 replicate DMA of the one materialized interior row
        def row(eng, p0, kmid):
            if eng is nc.scalar:
                eng.activation(pat[:, p0 + 1:p0 + 111], bc2(kmid, 110),
                               AF.Copy)
            else:
                eng.tensor_copy(pat[:, p0 + 1:p0 + 111], bc2(kmid, 110))
            nc.vector.tensor_copy(
                bass.AP(pat.tensor, pat[:, p0:p0 + 1].offset,
                        [list(pat.ap[0]), [111, 2]]),
                bass.AP(val.tensor, vofs(kmid - 1),
                        [list(val.ap[0]), [2, 2]]))

        row(nc.vector, 0, 1)
        row(nc.vector, 112, 4)
        row(nc.vector, co, 7)
        rsrc = bass.AP(pat.tensor, pat[:, 112:113].offset,
                       [list(pat.ap[0]), [0, IBLK - 1], [1, 112]])
        rdst = pat[:, 224:112 + IW].rearrange("p (r q) -> p r q", q=112)
        nc.gpsimd.dma_start(rdst, rsrc)

        # ---- output DMAs: rows 0..10, 11..109 (9x repeat of the 11-row
        # interior block), 110..111.  weng picks the ring (gpsimd SWDGE
        # normally; the last pair splits over the idle sync ring too).
        orows = out[128 * s:128 * (s + 1), :]
        src2 = pat[:, 112:112 + IW].rearrange("p (r q) -> p r q", r=1)
        if weng is None:
            half = 4 * IW
            src_h = bass.AP(src2.tensor, src2.offset,
                            [list(src2.ap[0]), [0, 4], [1, IW]])
            dst_h = orows[:, 1232:1232 + half].rearrange(
                "c (r q) -> c r q", q=IW)
            nc.gpsimd.dma_start(orows[:, 0:1232], pat[:, 0:1232])
            nc.gpsimd.dma_start(dst_h, src_h)
            src_t = bass.AP(src2.tensor, src2.offset,
                            [list(src2.ap[0]), [0, 5], [1, IW]])
            dst_t = orows[:, 1232 + half:1232 + 9 * IW].rearrange(
                "c (r q) -> c r q", q=IW)
            nc.sync.dma_start(dst_t, src_t)
            nc.sync.dma_start(orows[:, 12320:12544], pat[:, 1232:1456])
        else:
            weng.dma_start(orows[:, 0:1232], pat[:, 0:1232])
            srcr = bass.AP(src2.tensor, src2.offset,
                           [list(src2.ap[0]), [0, NRI - 1], [1, IW]])
            dstr = orows[:, 1232:1232 + 9 * IW].rearrange(
                "c (r q) -> c r q", q=IW)
            weng.dma_start(dstr, srcr)
            weng.dma_start(orows[:, 12320:12544], pat[:, 1232:1456])

    # pipeline: reads free-run on the sync ring (3 xt buffers); the SNN
    # runs batched for pairs {0,1} then solo for 2 and 3 (so pair 2's
    # write is not gated on pair 3's sums); patterns+writes trail so the
    # gpsimd ring only ever carries pattern copies + output DMAs
    stage_read(0)
    stage_read(1)
    stage_read(2)
    stage_read(3)
    stage_group(0, [0, 1])
    stage_pat(0, nc.gpsimd)
    stage_pat(1, nc.gpsimd)
    stage_group(1, [2, 3])
    stage_pat(2, nc.gpsimd)
    stage_pat(3, None)


def _build(dvals):
    import concourse.tile as tile
    from concourse import bacc, mybir
    from contextlib import ExitStack

    f32 = mybir.dt.float32
    f16 = mybir.dt.float16
    u8 = mybir.dt.uint8
    nc = bacc.Bacc("TRN2", target_bir_lowering=False, debug=False,
                   num_devices=NCORES)
    aps = {
        "xs": nc.dram_tensor("xs", [ROWS, HW], f16, kind="ExternalInput").ap(),
        "w_in_dup": nc.dram_tensor("w_in_dup", [128, HID], f32, kind="ExternalInput").ap(),
        "b1row": nc.dram_tensor("b1row", [1, HID], f32, kind="ExternalInput").ap(),
        "w_out4": nc.dram_tensor("w_out4", [128, 512], f32, kind="ExternalInput").ap(),
        "scale128": nc.dram_tensor("scale128", [128, 1], f32, kind="ExternalInput").ap(),
        "s2b2": nc.dram_tensor("s2b2", [128, 18], f32, kind="ExternalInput").ap(),
        "ident4": nc.dram_tensor("ident4", [4, 4], f32, kind="ExternalInput").ap(),
        "out": nc.dram_tensor("out", [ROWS, HW], u8, kind="ExternalOutput").ap(),
    }
    with tile.TileContext(nc) as tc:
        with ExitStack() as ctx:
            tc._emit_ctx = ctx
            _emit(tc, aps, dvals)
    nc.compile()
    return nc


def _host_params(w_in, bn1_gamma, bn1_beta, bn1_mean, bn1_var, decay_param,
                 v_th, w_out, conv_w, bn2_gamma, bn2_beta, bn2_mean, bn2_var,
                 scale):
    f32 = np.float32
    g1 = (bn1_gamma / np.sqrt(bn1_var + BN_EPS)).astype(f32)          # (HID,)
    b1 = (bn1_beta - bn1_mean * g1).astype(f32)                        # (HID,)
    # w_in (scaled, mean/HW folded) duplicated on both partition halves so
    # the per-sample K=64 matmuls read lhsT/rhs from matching partitions
    w_in_half = (w_in * (g1 / HW)[:, None]).T.astype(f32)              # (C, HID)
    w_in_dup = np.concatenate([w_in_half, w_in_half], axis=0)          # (128, HID)
    b1row = b1.reshape(1, HID)

    w_outT = np.ascontiguousarray(w_out.T.astype(f32))                 # (HID, C)
    # block-diagonal layout for the (128,1) pair matmul:
    # cols [0:128]=top chunk0, [128:256]=top chunk1, [256:384]=bot chunk0,
    # [384:512]=bot chunk1;  top feeds partitions 0..63 (even sample),
    # bot feeds partitions 64..127 (odd sample)
    w_out4 = np.zeros((128, 512), f32)
    w_out4[:, 0:64] = w_outT[0:128]
    w_out4[:, 128:192] = w_outT[128:256]
    w_out4[:, 320:384] = w_outT[0:128]
    w_out4[:, 448:512] = w_outT[128:256]

    # window sums of conv_w over valid 3x3 sub-windows
    k = conv_w.reshape(C, 3, 3).astype(f32)
    rsel = [(1, 3), (0, 3), (0, 2)]   # image row 0 / interior / row 111
    S = np.empty((C, 3, 3), f32)
    for a, (r0, r1) in enumerate(rsel):
        for ss, (c0, c1) in enumerate(rsel):
            S[:, a, ss] = k[:, r0:r1, c0:c1].sum(axis=(1, 2))
    g2 = (bn2_gamma / np.sqrt(bn2_var + BN_EPS)).astype(f32)           # (C,)
    b2 = (bn2_beta - bn2_mean * g2).astype(f32)
    S2g = S.reshape(C, 9) * g2[:, None]
    # val' = tanh(t1*(0.5*S2g) + (S2g + B2)); cols [0:9]=0.5*S2g,
    # [9:18]=S2g+B2
    s2b2_64 = np.empty((C, 18), f32)
    s2b2_64[:, 0:9] = 0.5 * S2g
    s2b2_64[:, 9:18] = S2g + b2[:, None]
    s2b2 = np.concatenate([s2b2_64, s2b2_64], axis=0)                  # (128,18)

    scale128 = np.concatenate([scale, scale]).astype(f32).reshape(128, 1)

    d = 1.0 / (1.0 + np.exp(-np.float64(decay_param)))

    return {
        "__dvals__": {"d": float(f32(d)), "vth": float(f32(v_th))},
        "w_in_dup": w_in_dup,
        "b1row": b1row,
        "w_out4": w_out4,
        "scale128": scale128,
        "s2b2": s2b2,
        "ident4": np.eye(4, dtype=f32),
    }


def kernel(**inputs):
    global LAST_RESULTS
    _ensure_ntff_hook_module()
    from concourse.bass_utils import run_bass_kernel_spmd

    x = np.asarray(inputs["x"], dtype=np.float32)
    params = _host_params(
        **{k: np.asarray(v) for k, v in inputs.items() if k != "x"})
    dvals = params.pop("__dvals__")

    key = ("nc", dvals["d"], dvals["vth"])
    if key not in _CACHE:
        _CACHE[key] = _build(dvals)
    nc = _CACHE[key]

    x_flat = np.ascontiguousarray(
        x.reshape(B * C, HW).astype(np.float16))
    in_maps = []
    for k in range(NCORES):
        m = dict(params)
        m["xs"] = x_flat[ROWS * k:ROWS * (k + 1)]
        in_maps.append(m)

    trace = bool(os.environ.get("KERNEL_TRACE"))
    res = run_bass_kernel_spmd(nc, in_maps, list(range(NCORES)), trace=trace)
    LAST_RESULTS = res
    out = np.concatenate([r["out"] for r in res.results], axis=0)
    # dequantize: u8 holds round(255*relu(tanh(.))), out = 1 + 0.25*rt
    out = 1.0 + out.astype(np.float32) * np.float32(0.25 / 255.0)
    return out.reshape(B, C, H, W)

